# revision 24
# baseline (speedup 1.0000x reference)
"""Distributed Bass kernel for GQA attention (nn_Attention_71536975282525).

Sharding: tensor-parallel over heads across 8 cores. Core c owns kv-head c and
q-heads 4c..4c+3. Inside each core:
  phase 1: q/k/v projections (head-dim-major), RoPE via stream_shuffle pair-swap
  phase 1b: PE-transposes for v (PV lhsT + cache_v) and rope'd k (cache_k)
  phase 2: causal attention with transposed scores (scores^T = K^T.T @ Q^T),
           exp on ACT, column sums via ones-matmul, PV accumulation
  phase 3: chunked AllGather of attn output + output projection column block
Host: transposes/casts x to bf16, slices weight shards, builds rope tables and
causal diagonal masks, assembles full outputs (column concat - no host math).
"""

import math
import os
import sys

import numpy as np

try:
    import concourse.bass as bass
except ImportError:
    sys.path.insert(0, "/opt/trn_rl_repo")
    import concourse.bass as bass

import concourse.mybir as mybir
import concourse.tile as tile
from concourse import bacc
from concourse.bass import ds, ts
from concourse.masks import make_identity

import ml_dtypes

BF16 = ml_dtypes.bfloat16
S = 2048
DIM = 4096
HD = 128
N_CORES = 8
QH = 4              # q heads per core
CH = 512            # seq chunk
NCHUNK = S // CH    # 4
NK = DIM // 128     # 32 contraction tiles
INV_SQRT_HD = 1.0 / math.sqrt(HD)
SWAP_MASK = [i ^ 1 for i in range(32)]

FP32 = mybir.dt.float32
BF = mybir.dt.bfloat16


def build():
    nc = bacc.Bacc(None, target_bir_lowering=False)

    xt = nc.declare_dram_parameter("xt", [DIM, S], BF, isOutput=False)
    wq = nc.declare_dram_parameter("wq", [DIM, QH * HD], BF, isOutput=False)
    wk = nc.declare_dram_parameter("wk", [DIM, HD], BF, isOutput=False)
    wv = nc.declare_dram_parameter("wv", [DIM, HD], BF, isOutput=False)
    wo = nc.declare_dram_parameter("wo", [DIM, CH], BF, isOutput=False)
    ctab = nc.declare_dram_parameter("ctab", [HD, S], BF, isOutput=False)
    stab = nc.declare_dram_parameter("stab", [HD, S], BF, isOutput=False)
    dmask = nc.declare_dram_parameter("dmask", [128, CH], BF, isOutput=False)

    out_blk = nc.declare_dram_parameter("out_blk", [S, CH], FP32, isOutput=True)
    ck_out = nc.declare_dram_parameter("ck", [S, HD], FP32, isOutput=True)
    cv_out = nc.declare_dram_parameter("cv", [S, HD], FP32, isOutput=True)

    rg = [list(range(N_CORES))]

    with tile.TileContext(nc) as tc:
        from contextlib import ExitStack

        with ExitStack() as ctx:
            # ---------------- persistent SBUF ----------------
            pers = ctx.enter_context(tc.tile_pool(name="pers", bufs=1))
            c_sb = pers.tile([128, S], BF, tag="ctab")
            s_sb = pers.tile([128, S], BF, tag="stab")
            m_sb = pers.tile([128, CH], BF, tag="dmask")
            ident = pers.tile([128, 128], BF, tag="ident")
            ones_k = pers.tile([128, 1], BF, tag="ones_k")
            ones_m = pers.tile([33, 128], FP32, tag="ones_m")
            q_rot = [pers.tile([128, S], BF, tag=f"q_rot{h}", name=f"q_rot{h}")
                     for h in range(QH)]
            k_rot = pers.tile([128, S], BF, tag="k_rot")
            v_seq = pers.tile([128, S], BF, tag="v_seq")
            wo_sb = pers.tile([128, NK * CH], BF, tag="wo")

            accp = ctx.enter_context(
                tc.tile_pool(name="acc_ps", bufs=2, space="PSUM"))
            scp = ctx.enter_context(
                tc.tile_pool(name="sc_ps", bufs=2, space="PSUM"))
            pvp = ctx.enter_context(
                tc.tile_pool(name="pv_ps", bufs=2, space="PSUM"))
            sbp = ctx.enter_context(
                tc.tile_pool(name="sb_ps", bufs=2, space="PSUM"))
            ptp = ctx.enter_context(tc.tile_pool(name="pt", bufs=4))
            osp = ctx.enter_context(tc.tile_pool(name="osml", bufs=2))
            drp = ctx.enter_context(
                tc.tile_pool(name="dram", bufs=1, space="DRAM"))

            phase_a = ExitStack()
            wqp = phase_a.enter_context(tc.tile_pool(name="wqkv", bufs=1))
            wq_sb = wqp.tile([128, NK * QH * HD], BF, tag="wq")
            wk_sb = wqp.tile([128, NK * HD], BF, tag="wk")
            wv_sb = wqp.tile([128, NK * HD], BF, tag="wv")
            xsp = phase_a.enter_context(tc.tile_pool(name="xs", bufs=6))
            rtp = phase_a.enter_context(tc.tile_pool(name="rtmp", bufs=3))
            tps = phase_a.enter_context(tc.tile_pool(name="tp_sb", bufs=4))

            def load_stripe(n):
                quarters = []
                for hh in range(4):
                    xh = xsp.tile([128, 8 * CH], BF, tag="xs",
                                  name=f"xs{n}_{hh}")
                    sr = xt[hh * 1024:(hh + 1) * 1024, ts(n, CH)]
                    nc.sync.dma_start(
                        xh[:].rearrange("p (k j) -> p k j", k=8),
                        sr.rearrange("(k p) j -> p k j", p=128),
                    )
                    quarters.append(xh)
                return quarters

            xq0 = xsp.tile([128, 8 * CH], BF, tag="xs", name="xs0_0")
            nc.sync.dma_start(
                xq0[:].rearrange("p (k j) -> p k j", k=8),
                xt[0:1024, ts(0, CH)].rearrange("(k p) j -> p k j", p=128))
            nc.sync.dma_start(wv_sb[:].rearrange("p (k m) -> p k m", k=NK),
                              wv.rearrange("(k p) m -> p k m", p=128))
            nc.sync.dma_start(
                wq_sb[:, 0: 8 * QH * HD].rearrange("p (k m) -> p k m", k=8),
                wq[0:1024, :].rearrange("(k p) m -> p k m", p=128))
            stripe0 = [xq0]
            for hh in range(1, 4):
                xh = xsp.tile([128, 8 * CH], BF, tag="xs", name=f"xs0_{hh}")
                nc.sync.dma_start(
                    xh[:].rearrange("p (k j) -> p k j", k=8),
                    xt[hh * 1024:(hh + 1) * 1024, ts(0, CH)]
                    .rearrange("(k p) j -> p k j", p=128))
                stripe0.append(xh)
            for qq in range(1, 4):
                nc.sync.dma_start(
                    wq_sb[:, qq * 8 * QH * HD: (qq + 1) * 8 * QH * HD]
                    .rearrange("p (k m) -> p k m", k=8),
                    wq[qq * 1024:(qq + 1) * 1024, :]
                    .rearrange("(k p) m -> p k m", p=128))
            nc.sync.dma_start(wk_sb[:].rearrange("p (k m) -> p k m", k=NK),
                              wk.rearrange("(k p) m -> p k m", p=128))
            nc.sync.dma_start(c_sb[:], ctab[:, :])
            nc.sync.dma_start(s_sb[:], stab[:, :])
            nc.sync.dma_start(m_sb[:], dmask[:, :])
            make_identity(nc, ident[:])
            nc.vector.memset(ones_k[:], 1.0)
            nc.vector.memset(ones_m[:], 1.0)
            nc.sync.dma_start(wo_sb[:].rearrange("p (k m) -> p k m", k=NK),
                              wo.rearrange("(k p) m -> p k m", p=128))

            ag_in = [drp.tile([QH * HD, CH], BF, tag=f"ag_in{c}",
                              name=f"ag_in{c}") for c in range(NCHUNK)]
            ag_out = [drp.tile([DIM, CH], BF, tag=f"ag_out{c}",
                               name=f"ag_out{c}", addr_space="Shared")
                      for c in range(NCHUNK)]
            # warmup collective, same shape as the real ones: absorbs the
            # cold-start cost of the CC path (first AG otherwise ~3-5x slower)
            wu_in = drp.tile([QH * HD, CH], BF, tag="wu_in")
            wu_out = drp.tile([DIM, CH], BF, tag="wu_out", addr_space="Shared")
            for h in range(QH):
                nc.sync.dma_start(wu_in[ts(h, 128), :], m_sb[:])
            nc.gpsimd.collective_compute(
                "AllGather", mybir.AluOpType.bypass, replica_groups=rg,
                ins=[wu_in.opt()], outs=[wu_out.opt()])

            def proj_chunk(n, halves):
                def proj(dst_slice, w_sb, m_off, m_stride, do_rope):
                    ps = accp.tile([128, CH], FP32, tag="acc",
                                   name=f"ps{n}_{m_off}_{m_stride}")
                    for kk in range(NK):
                        lhsT = w_sb[:, kk * m_stride + m_off:
                                    kk * m_stride + m_off + 128]
                        rhs = halves[kk // 8][:, ts(kk % 8, CH)]
                        nc.tensor.matmul(ps[:], lhsT, rhs,
                                         start=(kk == 0), stop=(kk == NK - 1))
                    if not do_rope:
                        nc.scalar.activation(dst_slice, ps[:],
                                             mybir.ActivationFunctionType.Copy)
                        return
                    raw = rtp.tile([128, CH], BF, tag="raw")
                    nc.scalar.activation(raw[:], ps[:],
                                         mybir.ActivationFunctionType.Copy)
                    shuf = rtp.tile([128, CH], BF, tag="shuf")
                    nc.vector.stream_shuffle(shuf[:], raw[:], SWAP_MASK)
                    nc.vector.tensor_mul(raw[:], raw[:], c_sb[:, ts(n, CH)])
                    nc.vector.tensor_mul(shuf[:], shuf[:], s_sb[:, ts(n, CH)])
                    nc.vector.tensor_add(dst_slice, raw[:], shuf[:])

                def qk_projs():
                    for h in range(QH):
                        proj(q_rot[h][:, ts(n, CH)], wq_sb, h * 128,
                             QH * HD, True)
                    proj(k_rot[:, ts(n, CH)], wk_sb, 0, HD, True)

                if n != 0:
                    qk_projs()
                vps = accp.tile([128, CH], FP32, tag="acc", name=f"vps{n}")
                for kk in range(NK):
                    nc.tensor.matmul(vps[:], wv_sb[:, ts(kk, HD)],
                                     halves[kk // 8][:, ts(kk % 8, CH)],
                                     start=(kk == 0), stop=(kk == NK - 1))
                vraw = rtp.tile([128, CH], BF, tag="vraw")
                nc.scalar.activation(vraw[:], vps[:],
                                     mybir.ActivationFunctionType.Copy)
                if n == 0:
                    qk_projs()

                for tt in range(4):
                    t = n * 4 + tt
                    nc.sync.dma_start(v_seq[:, ts(t, 128)],
                                      vraw[:, ts(tt, 128)], transpose=True)
                    nc.gpsimd.dma_start(cv_out[ts(t, 128), :],
                                        v_seq[:, ts(t, 128)])
                    ksq = tps.tile([128, 128], BF, tag="ksq")
                    nc.sync.dma_start(ksq[:], k_rot[:, ts(t, 128)],
                                      transpose=True)
                    nc.gpsimd.dma_start(ck_out[ts(t, 128), :], ksq[:])

            def attention_chunk(cq):
                nt = 4 * cq + 4
                for hp in range(2):
                    heads = (2 * hp, 2 * hp + 1)
                    pv = [pvp.tile([128, CH], FP32, tag="pv",
                                   name=f"pv{cq}_{hp}_{i}") for i in range(2)]
                    sums = sbp.tile([128, CH], FP32, tag="sb")
                    def qk_exp(t):
                        j = t - 4 * cq
                        off = 128 * j if j > 0 else 0
                        w = CH - off
                        sc = [scp.tile([128, CH], FP32, tag="sc",
                                       name=f"sc{cq}_{hp}_{t}_{i}")
                              for i in range(2)]
                        p_bf = [ptp.tile([128, CH], BF, tag="p",
                                         name=f"p{cq}_{hp}_{t}_{i}")
                                for i in range(2)]
                        for i, h in enumerate(heads):
                            nc.tensor.matmul(
                                sc[i][:, 0:w], k_rot[:, ts(t, 128)],
                                q_rot[h][:, ds(cq * CH + off, w)],
                                start=True, stop=True)
                        for i in range(2):
                            nc.scalar.activation(
                                p_bf[i][:, 0:w], sc[i][:, 0:w],
                                mybir.ActivationFunctionType.Exp,
                                scale=INV_SQRT_HD)
                            if j >= 0:
                                nc.vector.tensor_mul(p_bf[i][:, 0:w],
                                                     p_bf[i][:, 0:w],
                                                     m_sb[:, 0:w])
                        return p_bf

                    def pv_sums(t, p_bf):
                        j = t - 4 * cq
                        off = 128 * j if j > 0 else 0
                        w = CH - off
                        for i in range(2):
                            nc.tensor.matmul(
                                pv[i][:, ds(off, w)], v_seq[:, ts(t, 128)],
                                p_bf[i][:, 0:w],
                                start=(t == 0), stop=(t == nt - 1))
                        for i in range(2):
                            nc.tensor.matmul(
                                sums[32 * i:32 * i + 1, ds(off, w)],
                                ones_k[:], p_bf[i][:, 0:w],
                                start=(t == 0), stop=(t == nt - 1),
                                tile_position=(0, 32 * i))

                    prev = qk_exp(0)
                    for t in range(1, nt):
                        cur = qk_exp(t)
                        pv_sums(t - 1, prev)
                        prev = cur
                    pv_sums(nt - 1, prev)
                    for i, h in enumerate(heads):
                        s_sb2 = osp.tile([33, CH], FP32, tag="s_sb",
                                         name=f"s_sb{cq}_{hp}_{i}")
                        nc.vector.tensor_copy(s_sb2[32 * i:32 * i + 1, :],
                                              sums[32 * i:32 * i + 1, :])
                        bc = sbp.tile([128, CH], FP32, tag="sb",
                                      name=f"bc{cq}_{hp}_{i}")
                        nc.tensor.matmul(
                            bc[:], ones_m[32 * i:32 * i + 1, :],
                            s_sb2[32 * i:32 * i + 1, :],
                            start=True, stop=True)
                        r128 = osp.tile([128, CH], FP32, tag="r128")
                        nc.vector.reciprocal(r128[:], bc[:])
                        o_bf = osp.tile([128, CH], BF, tag="o_bf")
                        nc.vector.tensor_mul(o_bf[:], pv[i][:], r128[:])
                        nc.sync.dma_start(ag_in[cq][ts(h, 128), :], o_bf[:])

            def allgather_chunk(cq):
                nc.gpsimd.collective_compute(
                    "AllGather", mybir.AluOpType.bypass, replica_groups=rg,
                    ins=[ag_in[cq].opt()], outs=[ag_out[cq].opt()])

            g_tiles = {}

            def load_g(cq):
                g = gtp.tile([128, NK * CH], BF, tag="g", name=f"g{cq}")
                nc.sync.dma_start(
                    g[:].rearrange("p (k j) -> p k j", k=NK),
                    ag_out[cq].rearrange("(k p) j -> p k j", p=128),
                )
                g_tiles[cq] = g

            def wo_chunk(cq):
                g = g_tiles[cq]
                for st in range(4):
                    ops = accp.tile([128, CH], FP32, tag="acc",
                                    name=f"wops{cq}_{st}")
                    for kk in range(NK):
                        lhsT = g[:, kk * CH + st * 128: kk * CH + (st + 1) * 128]
                        nc.tensor.matmul(ops[:], lhsT, wo_sb[:, ts(kk, CH)],
                                         start=(kk == 0), stop=(kk == NK - 1))
                    osb = oop.tile([128, CH], FP32, tag="osb")
                    nc.vector.tensor_copy(osb[:], ops[:])
                    nc.sync.dma_start(out_blk[ds(cq * CH + st * 128, 128), :],
                                      osb[:])

            proj_chunk(0, stripe0)
            proj_chunk(1, load_stripe(1))
            attention_chunk(0)
            allgather_chunk(0)
            proj_chunk(2, load_stripe(2))
            attention_chunk(1)
            allgather_chunk(1)
            proj_chunk(3, load_stripe(3))
            phase_a.close()
            gtp = ctx.enter_context(tc.tile_pool(name="gth", bufs=3))
            oop = ctx.enter_context(tc.tile_pool(name="oout", bufs=3))
            load_g(0)
            attention_chunk(2)
            allgather_chunk(2)
            load_g(1)
            wo_chunk(0)
            attention_chunk(3)
            allgather_chunk(3)
            load_g(2)
            load_g(3)
            wo_chunk(1)
            wo_chunk(2)
            wo_chunk(3)

    if not nc.is_finalized():
        nc.finalize()
    return nc


_CACHED = {}


def _patch_ldw_opt():
    if _CACHED.get("ldw_patched") or os.environ.get("KERNEL_NO_LDW_OPT"):
        return
    from concourse import bass_utils as bu
    orig = bu.run_command

    def patched(cmd, **kw):
        if isinstance(cmd, list):
            cmd = ["--enable-ldw-opt=true" if c == "--enable-ldw-opt=false"
                   else c for c in cmd]
        return orig(cmd, **kw)

    bu.run_command = patched
    _CACHED["ldw_patched"] = True


def _prep(inputs):
    x = np.asarray(inputs["x"])[0]                       # [2048, 4096] f32
    wq = np.asarray(inputs["wq"])
    wk = np.asarray(inputs["wk"])
    wv = np.asarray(inputs["wv"])
    wo = np.asarray(inputs["wo"])
    cos = np.asarray(inputs["cos"])
    sin = np.asarray(inputs["sin"])

    xt = np.ascontiguousarray(x.T).astype(BF16)          # [4096, 2048]
    ct = np.empty((HD, S), np.float32)
    st = np.empty((HD, S), np.float32)
    for d in range(HD):
        ct[d] = cos[:, d // 2]
        st[d] = -sin[:, d // 2] if d % 2 == 0 else sin[:, d // 2]
    ct = ct.astype(BF16)
    st = st.astype(BF16)
    # diagonal masks: block j valid iff f >= p + 128*j
    f = np.arange(CH)[None, :]
    p = np.arange(128)[:, None]
    dm = (f >= p).astype(np.float32).astype(BF16)         # [128, 512] triangle

    in_maps = []
    for c in range(N_CORES):
        in_maps.append({
            "xt": xt,
            "wq": np.ascontiguousarray(wq[:, 512 * c:512 * (c + 1)]).astype(BF16),
            "wk": np.ascontiguousarray(wk[:, 128 * c:128 * (c + 1)]).astype(BF16),
            "wv": np.ascontiguousarray(wv[:, 128 * c:128 * (c + 1)]).astype(BF16),
            "wo": np.ascontiguousarray(wo[:, 512 * c:512 * (c + 1)]).astype(BF16),
            "ctab": ct, "stab": st, "dmask": dm,
        })
    return in_maps


def kernel(**inputs):
    from concourse.bass_utils import run_bass_kernel_spmd

    if "nc" not in _CACHED:
        _CACHED["nc"] = build()
    nc = _CACHED["nc"]
    in_maps = _prep(inputs)
    trace = bool(int(os.environ.get("KERNEL_TRACE", "0")))
    res = run_bass_kernel_spmd(nc, in_maps, core_ids=list(range(N_CORES)),
                               trace=trace)
    _CACHED["last_result"] = res
    outs = res.results

    out = np.empty((1, S, DIM), np.float32)
    ck = np.empty((1, S, N_CORES, HD), np.float32)
    cv = np.empty((1, S, N_CORES, HD), np.float32)
    for c in range(N_CORES):
        out[0, :, 512 * c:512 * (c + 1)] = outs[c]["out_blk"]
        ck[0, :, c, :] = outs[c]["ck"]
        cv[0, :, c, :] = outs[c]["cv"]
    return out, ck, cv


# revision 25
# speedup vs baseline: 1.0828x; 1.0828x over previous
"""Distributed Bass kernel for GQA attention (nn_Attention_71536975282525).

Sharding: tensor-parallel over heads across 8 cores. Core c owns kv-head c and
q-heads 4c..4c+3. Inside each core:
  phase 1: q/k/v projections (head-dim-major), RoPE via stream_shuffle pair-swap
  phase 1b: PE-transposes for v (PV lhsT + cache_v) and rope'd k (cache_k)
  phase 2: causal attention with transposed scores (scores^T = K^T.T @ Q^T),
           exp on ACT, column sums via ones-matmul, PV accumulation
  phase 3: chunked AllGather of attn output + output projection column block
Host: transposes/casts x to bf16, slices weight shards, builds rope tables and
causal diagonal masks, assembles full outputs (column concat - no host math).
"""

import math
import os
import sys

import numpy as np

try:
    import concourse.bass as bass
except ImportError:
    sys.path.insert(0, "/opt/trn_rl_repo")
    import concourse.bass as bass

import concourse.mybir as mybir
import concourse.tile as tile
from concourse import bacc
from concourse.bass import ds, ts
from concourse.masks import make_identity

import ml_dtypes

BF16 = ml_dtypes.bfloat16
S = 2048
DIM = 4096
HD = 128
N_CORES = 8
QH = 4              # q heads per core
CH = 512            # seq chunk
NCHUNK = S // CH    # 4
NK = DIM // 128     # 32 contraction tiles
INV_SQRT_HD = 1.0 / math.sqrt(HD)
SWAP_MASK = [i ^ 1 for i in range(32)]

FP32 = mybir.dt.float32
BF = mybir.dt.bfloat16


def build():
    nc = bacc.Bacc(None, target_bir_lowering=False)

    xt = nc.declare_dram_parameter("xt", [DIM, S], BF, isOutput=False)
    wq = nc.declare_dram_parameter("wq", [DIM, QH * HD], BF, isOutput=False)
    wk = nc.declare_dram_parameter("wk", [DIM, HD], BF, isOutput=False)
    wv = nc.declare_dram_parameter("wv", [DIM, HD], BF, isOutput=False)
    wo = nc.declare_dram_parameter("wo", [DIM, CH], BF, isOutput=False)
    ctab = nc.declare_dram_parameter("ctab", [HD, S], BF, isOutput=False)
    stab = nc.declare_dram_parameter("stab", [HD, S], BF, isOutput=False)
    dmask = nc.declare_dram_parameter("dmask", [128, CH], BF, isOutput=False)

    out_blk = nc.declare_dram_parameter("out_blk", [S, CH], FP32, isOutput=True)
    ck_out = nc.declare_dram_parameter("ck", [S, HD], FP32, isOutput=True)
    cv_out = nc.declare_dram_parameter("cv", [S, HD], FP32, isOutput=True)

    rg = [list(range(N_CORES))]

    with tile.TileContext(nc) as tc:
        from contextlib import ExitStack

        with ExitStack() as ctx:
            # ---------------- persistent SBUF ----------------
            pers = ctx.enter_context(tc.tile_pool(name="pers", bufs=1))
            c_sb = pers.tile([128, S], BF, tag="ctab")
            s_sb = pers.tile([128, S], BF, tag="stab")
            m_sb = pers.tile([128, CH], BF, tag="dmask")
            ident = pers.tile([128, 128], BF, tag="ident")
            ones_k = pers.tile([128, 1], BF, tag="ones_k")
            ones_m = pers.tile([33, 128], FP32, tag="ones_m")
            q_rot = [pers.tile([128, S], BF, tag=f"q_rot{h}", name=f"q_rot{h}")
                     for h in range(QH)]
            k_rot = pers.tile([128, S], BF, tag="k_rot")
            v_seq = pers.tile([128, S], BF, tag="v_seq")
            wo_sb = pers.tile([128, NK * CH], BF, tag="wo")

            accp = ctx.enter_context(
                tc.tile_pool(name="acc_ps", bufs=2, space="PSUM"))
            scp = ctx.enter_context(
                tc.tile_pool(name="sc_ps", bufs=2, space="PSUM"))
            pvp = ctx.enter_context(
                tc.tile_pool(name="pv_ps", bufs=2, space="PSUM"))
            sbp = ctx.enter_context(
                tc.tile_pool(name="sb_ps", bufs=2, space="PSUM"))
            ptp = ctx.enter_context(tc.tile_pool(name="pt", bufs=4))
            osp = ctx.enter_context(tc.tile_pool(name="osml", bufs=2))
            drp = ctx.enter_context(
                tc.tile_pool(name="dram", bufs=1, space="DRAM"))

            phase_a = ExitStack()
            wqp = phase_a.enter_context(tc.tile_pool(name="wqkv", bufs=1))
            wq_sb = wqp.tile([128, NK * QH * HD], BF, tag="wq")
            wk_sb = wqp.tile([128, NK * HD], BF, tag="wk")
            wv_sb = wqp.tile([128, NK * HD], BF, tag="wv")
            xsp = phase_a.enter_context(tc.tile_pool(name="xs", bufs=6))
            rtp = phase_a.enter_context(tc.tile_pool(name="rtmp", bufs=3))
            tps = phase_a.enter_context(tc.tile_pool(name="tp_sb", bufs=4))

            def load_stripe(n):
                quarters = []
                for hh in range(4):
                    xh = xsp.tile([128, 8 * CH], BF, tag="xs",
                                  name=f"xs{n}_{hh}")
                    sr = xt[hh * 1024:(hh + 1) * 1024, ts(n, CH)]
                    nc.sync.dma_start(
                        xh[:].rearrange("p (k j) -> p k j", k=8),
                        sr.rearrange("(k p) j -> p k j", p=128),
                    )
                    quarters.append(xh)
                return quarters

            xq0 = xsp.tile([128, 8 * CH], BF, tag="xs", name="xs0_0")
            nc.sync.dma_start(
                xq0[:].rearrange("p (k j) -> p k j", k=8),
                xt[0:1024, ts(0, CH)].rearrange("(k p) j -> p k j", p=128))
            nc.sync.dma_start(wv_sb[:].rearrange("p (k m) -> p k m", k=NK),
                              wv.rearrange("(k p) m -> p k m", p=128))
            nc.sync.dma_start(
                wq_sb[:, 0: 8 * QH * HD].rearrange("p (k m) -> p k m", k=8),
                wq[0:1024, :].rearrange("(k p) m -> p k m", p=128))
            stripe0 = [xq0]
            for hh in range(1, 4):
                xh = xsp.tile([128, 8 * CH], BF, tag="xs", name=f"xs0_{hh}")
                nc.sync.dma_start(
                    xh[:].rearrange("p (k j) -> p k j", k=8),
                    xt[hh * 1024:(hh + 1) * 1024, ts(0, CH)]
                    .rearrange("(k p) j -> p k j", p=128))
                stripe0.append(xh)
            for qq in range(1, 4):
                nc.sync.dma_start(
                    wq_sb[:, qq * 8 * QH * HD: (qq + 1) * 8 * QH * HD]
                    .rearrange("p (k m) -> p k m", k=8),
                    wq[qq * 1024:(qq + 1) * 1024, :]
                    .rearrange("(k p) m -> p k m", p=128))
            nc.sync.dma_start(wk_sb[:].rearrange("p (k m) -> p k m", k=NK),
                              wk.rearrange("(k p) m -> p k m", p=128))
            nc.sync.dma_start(c_sb[:], ctab[:, :])
            nc.sync.dma_start(s_sb[:], stab[:, :])
            nc.sync.dma_start(m_sb[:], dmask[:, :])
            make_identity(nc, ident[:])
            nc.vector.memset(ones_k[:], 1.0)
            nc.vector.memset(ones_m[:], 1.0)
            nc.sync.dma_start(wo_sb[:].rearrange("p (k m) -> p k m", k=NK),
                              wo.rearrange("(k p) m -> p k m", p=128))

            ag_in = [drp.tile([QH * HD, CH], BF, tag=f"ag_in{c}",
                              name=f"ag_in{c}") for c in range(NCHUNK)]
            ag_out = [drp.tile([DIM, CH], BF, tag=f"ag_out{c}",
                               name=f"ag_out{c}", addr_space="Shared")
                      for c in range(NCHUNK)]
            # warmup collective, same shape as the real ones: absorbs the
            # cold-start cost of the CC path (first AG otherwise ~3-5x slower)
            wu_in = drp.tile([QH * HD, CH], BF, tag="wu_in")
            wu_out = drp.tile([DIM, CH], BF, tag="wu_out", addr_space="Shared")
            for h in range(QH):
                nc.sync.dma_start(wu_in[ts(h, 128), :], m_sb[:])
            nc.gpsimd.collective_compute(
                "AllGather", mybir.AluOpType.bypass, replica_groups=rg,
                ins=[wu_in.opt()], outs=[wu_out.opt()])

            def proj_chunk(n, halves):
                def proj(dst_slice, w_sb, m_off, m_stride, do_rope):
                    ps = accp.tile([128, CH], FP32, tag="acc",
                                   name=f"ps{n}_{m_off}_{m_stride}")
                    for kk in range(NK):
                        lhsT = w_sb[:, kk * m_stride + m_off:
                                    kk * m_stride + m_off + 128]
                        rhs = halves[kk // 8][:, ts(kk % 8, CH)]
                        nc.tensor.matmul(ps[:], lhsT, rhs,
                                         start=(kk == 0), stop=(kk == NK - 1))
                    if not do_rope:
                        nc.scalar.activation(dst_slice, ps[:],
                                             mybir.ActivationFunctionType.Copy)
                        return
                    raw = rtp.tile([128, CH], BF, tag="raw")
                    nc.scalar.activation(raw[:], ps[:],
                                         mybir.ActivationFunctionType.Copy)
                    shuf = rtp.tile([128, CH], BF, tag="shuf")
                    nc.vector.stream_shuffle(shuf[:], raw[:], SWAP_MASK)
                    nc.vector.tensor_mul(raw[:], raw[:], c_sb[:, ts(n, CH)])
                    nc.vector.tensor_mul(shuf[:], shuf[:], s_sb[:, ts(n, CH)])
                    nc.vector.tensor_add(dst_slice, raw[:], shuf[:])

                def qk_projs():
                    for h in range(QH):
                        proj(q_rot[h][:, ts(n, CH)], wq_sb, h * 128,
                             QH * HD, True)
                    proj(k_rot[:, ts(n, CH)], wk_sb, 0, HD, True)

                if n != 0:
                    qk_projs()
                vps = accp.tile([128, CH], FP32, tag="acc", name=f"vps{n}")
                for kk in range(NK):
                    nc.tensor.matmul(vps[:], wv_sb[:, ts(kk, HD)],
                                     halves[kk // 8][:, ts(kk % 8, CH)],
                                     start=(kk == 0), stop=(kk == NK - 1))
                vraw = rtp.tile([128, CH], BF, tag="vraw")
                nc.scalar.activation(vraw[:], vps[:],
                                     mybir.ActivationFunctionType.Copy)
                if n == 0:
                    qk_projs()

                for tt in range(4):
                    t = n * 4 + tt
                    pv_t = scp.tile([128, 128], BF, tag="sc",
                                    name=f"pvt{t}")
                    nc.tensor.transpose(pv_t[:], vraw[:, ts(tt, 128)], ident[:])
                    nc.vector.tensor_copy(v_seq[:, ts(t, 128)], pv_t[:])
                    nc.gpsimd.dma_start(cv_out[ts(t, 128), :],
                                        v_seq[:, ts(t, 128)])
                    pk_t = scp.tile([128, 128], BF, tag="sc",
                                    name=f"pkt{t}")
                    nc.tensor.transpose(pk_t[:], k_rot[:, ts(t, 128)], ident[:])
                    ksq = tps.tile([128, 128], BF, tag="ksq")
                    nc.vector.tensor_copy(ksq[:], pk_t[:])
                    nc.gpsimd.dma_start(ck_out[ts(t, 128), :], ksq[:])

            def attention_chunk(cq):
                nt = 4 * cq + 4
                for hp in range(2):
                    heads = (2 * hp, 2 * hp + 1)
                    pv = [pvp.tile([128, CH], FP32, tag="pv",
                                   name=f"pv{cq}_{hp}_{i}") for i in range(2)]
                    sums = sbp.tile([128, CH], FP32, tag="sb")
                    def qk_exp(t):
                        j = t - 4 * cq
                        off = 128 * j if j > 0 else 0
                        w = CH - off
                        sc = [scp.tile([128, CH], FP32, tag="sc",
                                       name=f"sc{cq}_{hp}_{t}_{i}")
                              for i in range(2)]
                        p_bf = [ptp.tile([128, CH], BF, tag="p",
                                         name=f"p{cq}_{hp}_{t}_{i}")
                                for i in range(2)]
                        for i, h in enumerate(heads):
                            nc.tensor.matmul(
                                sc[i][:, 0:w], k_rot[:, ts(t, 128)],
                                q_rot[h][:, ds(cq * CH + off, w)],
                                start=True, stop=True)
                        for i in range(2):
                            nc.scalar.activation(
                                p_bf[i][:, 0:w], sc[i][:, 0:w],
                                mybir.ActivationFunctionType.Exp,
                                scale=INV_SQRT_HD)
                            if j >= 0:
                                nc.vector.tensor_mul(p_bf[i][:, 0:w],
                                                     p_bf[i][:, 0:w],
                                                     m_sb[:, 0:w])
                        return p_bf

                    def pv_sums(t, p_bf):
                        j = t - 4 * cq
                        off = 128 * j if j > 0 else 0
                        w = CH - off
                        for i in range(2):
                            nc.tensor.matmul(
                                pv[i][:, ds(off, w)], v_seq[:, ts(t, 128)],
                                p_bf[i][:, 0:w],
                                start=(t == 0), stop=(t == nt - 1))
                        for i in range(2):
                            nc.tensor.matmul(
                                sums[32 * i:32 * i + 1, ds(off, w)],
                                ones_k[:], p_bf[i][:, 0:w],
                                start=(t == 0), stop=(t == nt - 1),
                                tile_position=(0, 32 * i))

                    prev = qk_exp(0)
                    for t in range(1, nt):
                        cur = qk_exp(t)
                        pv_sums(t - 1, prev)
                        prev = cur
                    pv_sums(nt - 1, prev)
                    for i, h in enumerate(heads):
                        s_sb2 = osp.tile([33, CH], FP32, tag="s_sb",
                                         name=f"s_sb{cq}_{hp}_{i}")
                        nc.vector.tensor_copy(s_sb2[32 * i:32 * i + 1, :],
                                              sums[32 * i:32 * i + 1, :])
                        bc = sbp.tile([128, CH], FP32, tag="sb",
                                      name=f"bc{cq}_{hp}_{i}")
                        nc.tensor.matmul(
                            bc[:], ones_m[32 * i:32 * i + 1, :],
                            s_sb2[32 * i:32 * i + 1, :],
                            start=True, stop=True)
                        r128 = osp.tile([128, CH], FP32, tag="r128")
                        nc.vector.reciprocal(r128[:], bc[:])
                        o_bf = osp.tile([128, CH], BF, tag="o_bf")
                        nc.vector.tensor_mul(o_bf[:], pv[i][:], r128[:])
                        nc.sync.dma_start(ag_in[cq][ts(h, 128), :], o_bf[:])

            def allgather_chunk(cq):
                nc.gpsimd.collective_compute(
                    "AllGather", mybir.AluOpType.bypass, replica_groups=rg,
                    ins=[ag_in[cq].opt()], outs=[ag_out[cq].opt()])

            g_tiles = {}

            def load_g(cq):
                g = gtp.tile([128, NK * CH], BF, tag="g", name=f"g{cq}")
                nc.sync.dma_start(
                    g[:].rearrange("p (k j) -> p k j", k=NK),
                    ag_out[cq].rearrange("(k p) j -> p k j", p=128),
                )
                g_tiles[cq] = g

            def wo_chunk(cq):
                g = g_tiles[cq]
                for st in range(4):
                    ops = accp.tile([128, CH], FP32, tag="acc",
                                    name=f"wops{cq}_{st}")
                    for kk in range(NK):
                        lhsT = g[:, kk * CH + st * 128: kk * CH + (st + 1) * 128]
                        nc.tensor.matmul(ops[:], lhsT, wo_sb[:, ts(kk, CH)],
                                         start=(kk == 0), stop=(kk == NK - 1))
                    osb = oop.tile([128, CH], FP32, tag="osb")
                    nc.vector.tensor_copy(osb[:], ops[:])
                    nc.sync.dma_start(out_blk[ds(cq * CH + st * 128, 128), :],
                                      osb[:])

            proj_chunk(0, stripe0)
            proj_chunk(1, load_stripe(1))
            attention_chunk(0)
            allgather_chunk(0)
            proj_chunk(2, load_stripe(2))
            attention_chunk(1)
            allgather_chunk(1)
            proj_chunk(3, load_stripe(3))
            phase_a.close()
            gtp = ctx.enter_context(tc.tile_pool(name="gth", bufs=3))
            oop = ctx.enter_context(tc.tile_pool(name="oout", bufs=3))
            load_g(0)
            attention_chunk(2)
            allgather_chunk(2)
            load_g(1)
            wo_chunk(0)
            attention_chunk(3)
            allgather_chunk(3)
            load_g(2)
            load_g(3)
            wo_chunk(1)
            wo_chunk(2)
            wo_chunk(3)

    if not nc.is_finalized():
        nc.finalize()
    return nc


_CACHED = {}


def _patch_ldw_opt():
    if _CACHED.get("ldw_patched") or os.environ.get("KERNEL_NO_LDW_OPT"):
        return
    from concourse import bass_utils as bu
    orig = bu.run_command

    def patched(cmd, **kw):
        if isinstance(cmd, list):
            cmd = ["--enable-ldw-opt=true" if c == "--enable-ldw-opt=false"
                   else c for c in cmd]
        return orig(cmd, **kw)

    bu.run_command = patched
    _CACHED["ldw_patched"] = True


def _prep(inputs):
    x = np.asarray(inputs["x"])[0]                       # [2048, 4096] f32
    wq = np.asarray(inputs["wq"])
    wk = np.asarray(inputs["wk"])
    wv = np.asarray(inputs["wv"])
    wo = np.asarray(inputs["wo"])
    cos = np.asarray(inputs["cos"])
    sin = np.asarray(inputs["sin"])

    xt = np.ascontiguousarray(x.T).astype(BF16)          # [4096, 2048]
    ct = np.empty((HD, S), np.float32)
    st = np.empty((HD, S), np.float32)
    for d in range(HD):
        ct[d] = cos[:, d // 2]
        st[d] = -sin[:, d // 2] if d % 2 == 0 else sin[:, d // 2]
    ct = ct.astype(BF16)
    st = st.astype(BF16)
    # diagonal masks: block j valid iff f >= p + 128*j
    f = np.arange(CH)[None, :]
    p = np.arange(128)[:, None]
    dm = (f >= p).astype(np.float32).astype(BF16)         # [128, 512] triangle

    in_maps = []
    for c in range(N_CORES):
        in_maps.append({
            "xt": xt,
            "wq": np.ascontiguousarray(wq[:, 512 * c:512 * (c + 1)]).astype(BF16),
            "wk": np.ascontiguousarray(wk[:, 128 * c:128 * (c + 1)]).astype(BF16),
            "wv": np.ascontiguousarray(wv[:, 128 * c:128 * (c + 1)]).astype(BF16),
            "wo": np.ascontiguousarray(wo[:, 512 * c:512 * (c + 1)]).astype(BF16),
            "ctab": ct, "stab": st, "dmask": dm,
        })
    return in_maps


def kernel(**inputs):
    from concourse.bass_utils import run_bass_kernel_spmd

    if "nc" not in _CACHED:
        _CACHED["nc"] = build()
    nc = _CACHED["nc"]
    in_maps = _prep(inputs)
    trace = bool(int(os.environ.get("KERNEL_TRACE", "0")))
    res = run_bass_kernel_spmd(nc, in_maps, core_ids=list(range(N_CORES)),
                               trace=trace)
    _CACHED["last_result"] = res
    outs = res.results

    out = np.empty((1, S, DIM), np.float32)
    ck = np.empty((1, S, N_CORES, HD), np.float32)
    cv = np.empty((1, S, N_CORES, HD), np.float32)
    for c in range(N_CORES):
        out[0, :, 512 * c:512 * (c + 1)] = outs[c]["out_blk"]
        ck[0, :, c, :] = outs[c]["ck"]
        cv[0, :, c, :] = outs[c]["cv"]
    return out, ck, cv
